# revision 49
# baseline (speedup 1.0000x reference)
"""BlockGRUCell Trainium2 kernel.

Computation (per reference):
  hx = concat([h, x], -1)                       # (B, 2048)
  gate[b, 192g+o] = sum_i hx[b, 128g+i] * W[g, o, i]   # block-diagonal matmul
  r, c, u = split(gate + bias, 3)               # bias == 0 from setup_inputs
  h_new = sigmoid(u) * tanh(sigmoid(r) * c) + (1 - sigmoid(u)) * h

Sharding: data-parallel over batch across 8 NeuronCores (2048 rows each),
weights replicated.

The TensorE matmul contracts over the partition dim, so the stationary
operand must be hx^T per 128-feature block. The host pre-packs hx into
per-tile transposed bf16 panels:
  hxt[t, p, 128g+b] = hx[128t+b, 128g+p]

Engine budget per 128-row tile (measured on HW; both compute engines run
~95% busy through the middle):
  - ACT: sigmoid(r), sigmoid(u) from PSUM (~0.92us each) + tanh from the
    rc-in-PSUM panel (~0.93us).  ACT is 1 elem/cycle/lane at 1.2 GHz
    regardless of dtype and is the only engine with transcendentals, so
    3 passes x 2.1M elems/core is a hard floor.  NOTE: these call times
    are SBUF-pressure-sensitive — adding [128,2048] SBUF tiles inflated
    every ACTIVATE ~20-45%; writing rc into the dead gR PSUM panel
    (below) deflated them ~15%.
  - DVE (the slightly longer pole): rc = sigmoid(r)*c reads PSUM fp32 ->
    stuck at 1x mode (~1.2us); the blend (d = cand-h, e = upd*d,
    hn = h+e) runs on fp16 SBUF tiles -> 2x_1P mode (~0.68us each).
  - DMA: hxt bf16 (512K) + h fp16 row-major (256K) + out fp16 (256K)
    ~= 1.03 MiB/tile, ~64% duty.  Weights and the first h pair ride the
    Scalar engine's HWDGE ring so its ~1.5us cold-start overlaps the
    Sync ring carrying hxt.
  - PE: 20 block matmuls, bf16; ~40% duty so HAM keeps it half-cold,
    which is harmless here.
h and the output travel as fp16 (11-bit mantissa): rounding error ~7e-4
relative, well inside the 2e-2 gate; halves both DMA streams and enables
the DVE 2x mode.  Wall ~72.5us = ~7us fixed NEFF preamble + ~5us
cold-DMA ramp + ACT/DVE-co-saturated middle + ~4us drain.  This layout
is a sharp optimum: 18 of 21 measured variants (PSUM repacking, call
merging, tile pairing, GpSimd offload, DMA-accum output, ring/chunk
re-choreography, buffer-depth and allocation changes) regressed or tied.
"""

import numpy as np
import ml_dtypes

import concourse.bass as bass
import concourse.bacc as bacc
import concourse.tile as tile
import concourse.mybir as mybir
from concourse.bass_utils import run_bass_kernel_spmd

N_CORES = 8
BATCH = 16384
BS = BATCH // N_CORES            # rows per core
P = 128
NT = BS // P                     # 128-row tiles per core
HID = 1024
G = 16                           # feature blocks
IN_PER = 128
OUT_PER = 192
GATE = 3 * HID                   # 3072
PSUM_BANK_F32 = 512

F32 = mybir.dt.float32
BF16 = mybir.dt.bfloat16
F16 = mybir.dt.float16
AFT = mybir.ActivationFunctionType
ALU = mybir.AluOpType


def _body(tc, nc, hxt_d, h_d, wt_d, out_d):
    with (
        tc.tile_pool(name="consts", bufs=1) as consts,
        tc.tile_pool(name="io", bufs=6) as io,
        tc.tile_pool(name="panels", bufs=4) as panels,
        tc.tile_pool(name="gatep", bufs=4, space="PSUM") as gatep,
    ):
        # the weights ride the Scalar engine's HWDGE queue: its ring pays the
        # ~1.5us cold-start in parallel with the Sync ring carrying hxt, and
        # ACT is idle until the first sigmoid anyway.  The first chunk is
        # exactly what sigmoid(r[0:512]) needs (wt cols 0-511), so tile 0's
        # first epilogue half starts as early as the cold DMA allows.
        wt_s = consts.tile([P, G * OUT_PER], BF16)
        nc.scalar.dma_start(out=wt_s[:, 0:GATE // 2], in_=wt_d[:, 0:GATE // 2])
        nc.scalar.dma_start(out=wt_s[:, GATE // 2:], in_=wt_d[:, GATE // 2:])
        # tile 0's h ride-along: the scalar ring is free after the weights,
        # and keeping this 512K off the sync ring lets hxt[1] start sooner
        h2_first = io.tile([P, 2 * HID], F16, tag="h2", bufs=4)
        nc.scalar.dma_start(out=h2_first, in_=h_d[0])

        # warm the sigmoid/tanh ACT table during the initial DMAs (the
        # ~2.7us ACT_TABLE_LOAD otherwise lands on tile 0's critical path)
        warm = consts.tile([P, 1], F32)
        nc.vector.memset(warm, 0.0)
        nc.scalar.activation(warm, warm, AFT.Sigmoid)

        h2 = None
        out2 = None
        for t in range(NT):
            hxt = io.tile([P, G * P], BF16, tag="hxt")
            if t == 0:
                # h-half first: the r-gate matmuls need only blocks 0-5
                nc.sync.dma_start(out=hxt[:, 0:G * P // 2],
                                  in_=hxt_d[0, :, 0:G * P // 2])
                nc.sync.dma_start(out=hxt[:, G * P // 2:],
                                  in_=hxt_d[0, :, G * P // 2:])
            else:
                nc.sync.dma_start(out=hxt, in_=hxt_d[t])
            if t % 2 == 0:
                # h arrives pair-packed fp16: one 512K DMA per two tiles
                # (the first pair's was issued on the scalar ring above)
                if t == 0:
                    h2 = h2_first
                else:
                    h2 = io.tile([P, 2 * HID], F16, tag="h2", bufs=4)
                    nc.sync.dma_start(out=h2, in_=h_d[t // 2])
                out2 = io.tile([P, 2 * HID], F16, tag="out2", bufs=4)
            h_t = h2[:, (t % 2) * HID:(t % 2 + 1) * HID]

            # gate panels = the r/c/u split exactly (2 PSUM banks each)
            gR = gatep.tile([P, HID], F32, tag="gate")
            gC = gatep.tile([P, HID], F32, tag="gate")
            gU = gatep.tile([P, HID], F32, tag="gate")
            gs = (gR, gC, gU)

            for g in range(G):
                lhsT = hxt[:, g * P:(g + 1) * P]
                w0 = g * OUT_PER
                # split matmul writes at PSUM bank (512) boundaries
                c0 = w0
                while c0 < w0 + OUT_PER:
                    c1 = min(w0 + OUT_PER,
                             (c0 // PSUM_BANK_F32 + 1) * PSUM_BANK_F32)
                    gate = gs[c0 // HID]
                    nc.tensor.matmul(gate[:, c0 % HID:(c0 % HID) + c1 - c0],
                                     lhsT, wt_s[:, c0:c1],
                                     start=True, stop=True)
                    c0 = c1

            reset = panels.tile([P, HID], F16, tag="reset")
            rc = panels.tile([P, HID], F16, tag="rc")
            cand = panels.tile([P, HID], F16, tag="cand")
            upd = panels.tile([P, HID], F16, tag="upd")
            dd = panels.tile([P, HID], F16, tag="dd")
            ee = panels.tile([P, HID], F16, tag="ee")
            hn = out2[:, (t % 2) * HID:(t % 2 + 1) * HID]

            # the first tile's epilogue runs in halves so ACT starts as soon
            # as the first blocks' matmuls land; the last two tiles run in
            # halves/quarters so their serial ACT<->DVE chain (fully exposed
            # at the end of the pipeline) drains finer-grained and the final
            # store streams out early
            if t == 0 or t == NT - 2:
                splits = [(0, HID // 2), (HID // 2, HID)]
            elif t == NT - 1:
                splits = [(HID * i // 4, HID * (i + 1) // 4) for i in range(4)]
            else:
                splits = [(0, HID)]
            # for unsplit (middle) tiles, rc overwrites the dead gR panel in
            # place: no extra SBUF tile, and tanh reads PSUM (init 172 vs
            # 224 cycles).  Split tiles keep the SBUF rc: in-place reuse
            # would let slice k's rc-write falsely order against slice
            # k+1's sigmoid read under whole-tile dependency tracking.
            rc_t = gR if len(splits) == 1 else rc
            for idx, (a, b) in enumerate(splits):
                nc.scalar.activation(reset[:, a:b], gR[:, a:b], AFT.Sigmoid)
                nc.vector.tensor_tensor(rc_t[:, a:b], gC[:, a:b],
                                        reset[:, a:b], ALU.mult)
                # mid-kernel: sigmoid(u) before tanh — ACT's queue is
                # strict FIFO and tanh waits on DVE's rc, so the
                # independent sigmoid avoids a head-of-line stall.  In the
                # drain (last two tiles) the fully-exposed chain runs
                # through tanh->blend, so there tanh goes first and
                # sigmoid(u) overlaps the blend's first op instead.
                if t < NT - 2:
                    nc.scalar.activation(upd[:, a:b], gU[:, a:b],
                                         AFT.Sigmoid)
                    nc.scalar.activation(cand[:, a:b], rc_t[:, a:b],
                                         AFT.Tanh)
                else:
                    nc.scalar.activation(cand[:, a:b], rc_t[:, a:b],
                                         AFT.Tanh)
                    nc.scalar.activation(upd[:, a:b], gU[:, a:b],
                                         AFT.Sigmoid)
                # h_new = h + upd*(cand - h)
                nc.vector.tensor_sub(dd[:, a:b], cand[:, a:b], h_t[:, a:b])
                nc.vector.tensor_mul(ee[:, a:b], upd[:, a:b], dd[:, a:b])
                nc.vector.tensor_add(hn[:, a:b], h_t[:, a:b], ee[:, a:b])
                if t == NT - 1:
                    # the final quarter's store rides the scalar ring,
                    # idle after the last activation
                    eng = nc.scalar if idx == len(splits) - 1 else nc.sync
                    lo = 0 if idx == 0 else HID + a
                    eng.dma_start(out=out_d[t // 2][:, lo:HID + b],
                                  in_=out2[:, lo:HID + b])
            if t % 2 == 1 and t != NT - 1:
                nc.sync.dma_start(out=out_d[t // 2], in_=out2)


_NC_CACHE = {}


def _build_nc():
    if "nc" in _NC_CACHE:
        return _NC_CACHE["nc"]
    nc = bacc.Bacc()
    hxt_d = nc.dram_tensor("hxt", [NT, P, G * P], BF16, kind="ExternalInput")
    h_d = nc.dram_tensor("h2", [NT // 2, P, 2 * HID], F16,
                         kind="ExternalInput")
    wt_d = nc.dram_tensor("wt", [P, G * OUT_PER], BF16, kind="ExternalInput")
    out_d = nc.dram_tensor("out", [NT // 2, P, 2 * HID], F16,
                           kind="ExternalOutput")
    with tile.TileContext(nc) as tc:
        _body(tc, nc, hxt_d, h_d, wt_d, out_d)
    nc.compile()
    _NC_CACHE["nc"] = nc
    return nc


def _np_reference(x, h, weight, bias):
    hx = np.concatenate([h, x], axis=-1)
    xg = hx.reshape(x.shape[0], G, IN_PER)
    gate = np.einsum("bgi,goi->bgo", xg, weight).reshape(x.shape[0], GATE)
    gate = gate + bias
    r, c, u = np.split(gate, 3, axis=-1)
    reset = 1.0 / (1.0 + np.exp(-r))
    cand = np.tanh(reset * c)
    upd = 1.0 / (1.0 + np.exp(-u))
    return (upd * cand + (1.0 - upd) * h).astype(np.float32)


def _pack_hxt(hs, xs):
    """-> [NT, 128, 2048] bf16 with hxt[t, p, 128g+b] = hx[128t+b, 128g+p],
    where hx = concat([h, x], -1) per-row (blocks 0-7 = h, 8-15 = x)."""
    def tp(a):                      # [BS, 1024] -> [NT, 128, 8, 128]
        return a.reshape(NT, P, 8, P).transpose(0, 3, 2, 1)   # [t, p, g, b]
    arr = np.concatenate([tp(hs), tp(xs)], axis=2)            # [t, p, 16, b]
    return np.ascontiguousarray(arr.reshape(NT, P, G * P)).astype(
        ml_dtypes.bfloat16)


def _pack_pairs(a):
    """[BS, 1024] -> [NT//2, 128, 2048] fp16
    with [q, p, 1024s+f] = a[256q+128s+p, f]."""
    return np.ascontiguousarray(
        a.reshape(NT // 2, 2, P, HID).transpose(0, 2, 1, 3)
        .reshape(NT // 2, P, 2 * HID)).astype(np.float16)


def _unpack_pairs(a):
    """inverse of _pack_pairs, back to fp32."""
    return np.ascontiguousarray(
        a.reshape(NT // 2, P, 2, HID).transpose(0, 2, 1, 3)
        .reshape(BS, HID)).astype(np.float32)


def _run(x, h, weight, bias, trace=False, tmpdir=None):
    # wt[p, 192g+o] = W[g, o, p] — the exact SBUF layout, one contiguous DMA
    wt = np.ascontiguousarray(
        weight.transpose(2, 0, 1).reshape(P, G * OUT_PER)).astype(
        ml_dtypes.bfloat16)
    nc = _build_nc()
    in_maps = []
    for c in range(N_CORES):
        sl = slice(c * BS, (c + 1) * BS)
        xs, hs = x[sl], h[sl]
        in_maps.append({
            "hxt": _pack_hxt(hs, xs),
            "h2": _pack_pairs(hs),
            "wt": wt,
        })
    res = run_bass_kernel_spmd(nc, in_maps, core_ids=list(range(N_CORES)),
                               trace=trace, tmpdir=tmpdir)
    out = np.concatenate([_unpack_pairs(m["out"]) for m in res.results],
                         axis=0)
    return out, res


def kernel(x, h, weight, bias):
    x = np.asarray(x, dtype=np.float32)
    h = np.asarray(h, dtype=np.float32)
    weight = np.asarray(weight, dtype=np.float32)
    bias = np.asarray(bias, dtype=np.float32)
    if np.any(bias != 0.0):
        # setup_inputs() always passes zero bias; keep a correct fallback.
        return _np_reference(x, h, weight, bias)
    out, _ = _run(x, h, weight, bias)
    return out
